# revision 11
# baseline (speedup 1.0000x reference)
"""Trainium2 Bass kernel for a convolutional GRU (nn_ConvolutionalRNN).

Reference semantics (per timestep t, torch-GRUCell-style with conv1d gates):
    gi = conv1d(x[t], w_ih) + b_ih          # [B, 3C, L]
    gh = conv1d(h,    w_hh) + b_hh          # [B, 3C, L]
    r = sigmoid(gi_r + gh_r); z = sigmoid(gi_z + gh_z)
    n = tanh(gi_n + r * gh_n)
    h = n + z * (h - n) = z*h + n*(1-z)
    ys[t] = h

Sharding: data-parallel over batch, B=16 over 8 cores -> BL=2 per core.

Kernel structure (per core):
  * The per-core batch (BL=2) is split into TWO independent recurrence
    chains (b=0, b=1) that interleave on the engines: while chain 0 runs
    its serial gate chain, chain 1's matmuls/activations fill the gaps.
  * SBUF activations are bf16 (DVE 2x perf mode); PSUM accumulates fp32.
  * h lives in `hh` [64, NB, b, 260] (written in place by the GRU update)
    and x[t] in a separate `xx` ring (DMA'd in).  Keeping them in
    separate tiles lets the x-side conv matmuls run ~a step early (off
    the critical path, keeping the PE warm) and avoids any false
    write-write coupling between the x-prefetch DMA and the h update.
  * Per gate bank the K=3 conv is 6 accumulating matmuls (3 x-taps with
    contraction 64 done early + 3 h-taps with contraction 64 on the
    critical path):
      RZ bank [z-gates; r-gates]
      NN bank [a_n = i_n + gh_n; gh_n] (x contributes only to cols 0:64)
  * Per chain elementwise:
      cc   = sigmoid(-RZ + bneg)            # ACT: [zc; rc], bf16
      u    = (NN[64:] + bhh_n) * cc[64:]    # DVE stt from PSUM
      npre = (NN[0:64] + b_n) - u           # DVE stt from PSUM
      n    = tanh(npre)                     # ACT
      z    = 1 - cc[0:64]                   # GpSimd
      zh   = z * h                          # GpSimd
      nzc  = n * cc[0:64]                   # DVE
      h'   = nzc + zh                       # DVE -> next hh buffer
"""

import numpy as np
import ml_dtypes
from contextlib import ExitStack

from concourse import bacc, mybir
import concourse.tile as tile
from concourse.bass_utils import run_bass_kernel_spmd

T, B, CIN, COUT, L = 128, 16, 64, 64, 256
GATES = 3 * COUT
NCORES = 8
BL = B // NCORES          # batch per core = 2 (one recurrence chain each)
NB = 4                    # h/x buffer ring depth
LPAD = 260                # 2 zero cols + 256 interior + 2 zero cols
ILO, IHI = 2, 258         # interior column range
F32 = mybir.dt.float32
BF16 = mybir.dt.bfloat16
AF = mybir.ActivationFunctionType
ALU = mybir.AluOpType
BF = ml_dtypes.bfloat16


def _build_nc():
    nc = bacc.Bacc(trn_type="TRN2", target_bir_lowering=False, debug=False)

    x_d = nc.dram_tensor("x", [T, CIN, BL, L], BF16, kind="ExternalInput").ap()
    h0_d = nc.dram_tensor("h0", [COUT, BL, L], BF16, kind="ExternalInput").ap()
    wrzh_d = nc.dram_tensor("wrzh", [64, 3, 128], BF16, kind="ExternalInput").ap()
    wrzx_d = nc.dram_tensor("wrzx", [64, 3, 128], BF16, kind="ExternalInput").ap()
    wnh_d = nc.dram_tensor("wnh", [64, 3, 128], BF16, kind="ExternalInput").ap()
    wnx_d = nc.dram_tensor("wnx", [64, 3, 128], BF16, kind="ExternalInput").ap()
    bneg_d = nc.dram_tensor("bneg", [128, 1], F32, kind="ExternalInput").ap()
    ben_d = nc.dram_tensor("ben", [128, 1], F32, kind="ExternalInput").ap()
    ys_d = nc.dram_tensor("ys", [T, COUT, BL, L], BF16, kind="ExternalOutput").ap()

    with tile.TileContext(nc) as tc, ExitStack() as ctx:
        persist = ctx.enter_context(tc.tile_pool(name="persist", bufs=1))
        work = ctx.enter_context(tc.tile_pool(name="work", bufs=2))
        ps_rz = [ctx.enter_context(tc.tile_pool(name=f"ps_rz{c}", bufs=1, space="PSUM"))
                 for c in range(BL)]
        ps_nn = [ctx.enter_context(tc.tile_pool(name=f"ps_nn{c}", bufs=1, space="PSUM"))
                 for c in range(BL)]

        # --- one-time setup -------------------------------------------------
        wrzh = persist.tile([64, 3, 128], BF16)
        wrzx = persist.tile([64, 3, 128], BF16)
        wnh = persist.tile([64, 3, 128], BF16)
        wnx = persist.tile([64, 3, 128], BF16)
        bneg = persist.tile([128, 1], F32)
        ben = persist.tile([128, 1], F32)
        nc.sync.dma_start(out=wrzh[:], in_=wrzh_d)
        nc.sync.dma_start(out=wrzx[:], in_=wrzx_d)
        nc.sync.dma_start(out=wnh[:], in_=wnh_d)
        nc.sync.dma_start(out=wnx[:], in_=wnx_d)
        nc.sync.dma_start(out=bneg[:], in_=bneg_d)
        nc.sync.dma_start(out=ben[:], in_=ben_d)

        hh = persist.tile([64, NB, BL, LPAD], BF16)
        xx = persist.tile([64, NB, BL, LPAD], BF16)
        nc.vector.memset(hh[:], 0.0)
        nc.vector.memset(xx[:], 0.0)
        nc.sync.dma_start(out=hh[:, 0, :, ILO:IHI], in_=h0_d)
        nc.sync.dma_start(out=xx[:, 0, :, ILO:IHI], in_=x_d[0])
        nc.sync.dma_start(out=xx[:, 1, :, ILO:IHI], in_=x_d[1])

        def psum_tiles(t):
            # [128, 512] fp32 = exactly one PSUM bank per tile (first 256
            # cols used) so no two chains ever share a bank.
            rz = [ps_rz[c].tile([128, 512], F32, tag=f"rz{c}",
                                name=f"rz{c}_{t}")[:, 0:L] for c in range(BL)]
            nn = [ps_nn[c].tile([128, 512], F32, tag=f"nn{c}",
                                name=f"nn{c}_{t}")[:, 0:L] for c in range(BL)]
            return rz, nn

        def x_taps(t, rz, nn):
            # x-side conv for step t: contraction 64, start=True clears the
            # bank.  Runs ~a step early (only needs the prefetched x and the
            # bank's release), keeping the PE busy through the gate chain.
            for c in range(BL):
                rhs = lambda k: xx[:, t % NB, c, 1 + k:1 + k + L]
                for k in range(3):
                    nc.tensor.matmul(rz[c][:], wrzx[:, k, :], rhs(k),
                                     start=(k == 0), stop=False)
                for k in range(3):
                    nc.tensor.matmul(nn[c][:], wnx[:, k, :], rhs(k),
                                     start=(k == 0), stop=False)

        rz_t, nn_t = psum_tiles(0)
        x_taps(0, rz_t, nn_t)

        # --- the recurrence -------------------------------------------------
        for t in range(T):
            buf, nbuf = t % NB, (t + 1) % NB
            if t + 2 < T:
                # prefetch distance 2 so the load never trails the chain
                nc.sync.dma_start(out=xx[:, (t + 2) % NB, :, ILO:IHI],
                                  in_=x_d[t + 2])

            rz, nn = rz_t, nn_t
            # h-side conv: on the critical path (needs h' of step t-1)
            for c in range(BL):
                rhs = lambda k: hh[:, buf, c, 1 + k:1 + k + L]
                for k in range(3):
                    nc.tensor.matmul(rz[c][:], wrzh[:, k, :], rhs(k),
                                     start=False, stop=(k == 2))
                for k in range(3):
                    nc.tensor.matmul(nn[c][:], wnh[:, k, :], rhs(k),
                                     start=False, stop=(k == 2))

            cc, u, npre, n, z, zh, nzc = ({} for _ in range(7))
            for c in range(BL):
                # ACT: sigma- per chain
                cc[c] = work.tile([128, L], BF16, tag=f"cc{c}", name=f"cc{c}_{t}")
                nc.scalar.activation(cc[c][:], rz[c][:], AF.Sigmoid,
                                     bias=bneg[:], scale=-1.0)

                # DVE; u and npre both read straight from PSUM so only the
                # sigma- is upstream of them on the critical chain
                u[c] = work.tile([64, L], BF16, tag=f"u{c}", name=f"u{c}_{t}")
                nc.vector.scalar_tensor_tensor(u[c][:], nn[c][64:128, :],
                                               ben[64:128], cc[c][64:128, :],
                                               op0=ALU.add, op1=ALU.mult)
                npre[c] = work.tile([64, L], BF16, tag=f"npre{c}", name=f"npre{c}_{t}")
                nc.vector.scalar_tensor_tensor(npre[c][:], nn[c][0:64, :],
                                               ben[0:64], u[c][:],
                                               op0=ALU.add, op1=ALU.subtract)

                n[c] = work.tile([64, L], BF16, tag=f"n{c}", name=f"n{c}_{t}")
                nc.scalar.activation(n[c][:], npre[c][:], AF.Tanh)

                # GpSimd (off the critical chain): z = 1-zc, zh = z*h
                z[c] = work.tile([64, L], BF16, tag=f"z{c}", name=f"z{c}_{t}")
                nc.gpsimd.tensor_scalar(z[c][:], cc[c][0:64, :], -1.0, 1.0,
                                        op0=ALU.mult, op1=ALU.add)
                zh[c] = work.tile([64, L], BF16, tag=f"zh{c}", name=f"zh{c}_{t}")
                nc.gpsimd.tensor_tensor(zh[c][:], z[c][:],
                                        hh[:, buf, c, ILO:IHI], op=ALU.mult)
                nzc[c] = work.tile([64, L], BF16, tag=f"nzc{c}", name=f"nzc{c}_{t}")
                nc.vector.tensor_mul(nzc[c][:], n[c][:], cc[c][0:64, :])
                nc.vector.tensor_add(hh[:, nbuf, c, ILO:IHI], nzc[c][:],
                                     zh[c][:])

            nc.sync.dma_start(out=ys_d[t], in_=hh[:, nbuf, :, ILO:IHI])

            if t + 1 < T:
                rz_t, nn_t = psum_tiles(t + 1)
                x_taps(t + 1, rz_t, nn_t)

    nc.compile()
    return nc


_NC = None


def _get_nc():
    global _NC
    if _NC is None:
        _NC = _build_nc()
    return _NC


def _prep_in_maps(x, h0, w_ih, w_hh, b_ih, b_hh):
    w_ih = np.asarray(w_ih, np.float32)   # [192, 64, 3] (r, z, n gates)
    w_hh = np.asarray(w_hh, np.float32)
    b = np.asarray(b_ih, np.float32) + np.asarray(b_hh, np.float32)
    bhh = np.asarray(b_hh, np.float32)

    # RZ weights: h-side (whh) and x-side (wih) as separate contraction-64
    # stationaries; cols 0:64 = z gates, 64:128 = r gates
    wrzh = np.zeros((64, 3, 128), np.float32)
    wrzx = np.zeros((64, 3, 128), np.float32)
    for k in range(3):
        wrzh[:, k, 0:64] = w_hh[64:128, :, k].T
        wrzh[:, k, 64:128] = w_hh[0:64, :, k].T
        wrzx[:, k, 0:64] = w_ih[64:128, :, k].T
        wrzx[:, k, 64:128] = w_ih[0:64, :, k].T
    # NN weights: cols 0:64 = a_n (h+x), cols 64:128 = gh_n (h only)
    wnh = np.zeros((64, 3, 128), np.float32)
    wnx = np.zeros((64, 3, 128), np.float32)
    for k in range(3):
        wnh[:, k, 0:64] = w_hh[128:192, :, k].T
        wnh[:, k, 64:128] = w_hh[128:192, :, k].T
        wnx[:, k, 0:64] = w_ih[128:192, :, k].T

    bneg = np.zeros((128, 1), np.float32)
    bneg[0:64, 0] = -b[64:128]   # -b_z
    bneg[64:128, 0] = -b[0:64]   # -b_r
    ben = np.zeros((128, 1), np.float32)
    ben[0:64, 0] = b[128:192]            # b_ih_n + b_hh_n
    ben[64:128, 0] = bhh[128:192]        # b_hh_n

    wrzh = wrzh.astype(BF)
    wrzx = wrzx.astype(BF)
    wnh = wnh.astype(BF)
    wnx = wnx.astype(BF)
    x = np.asarray(x, np.float32)
    h0 = np.asarray(h0, np.float32)
    in_maps = []
    for cr in range(NCORES):
        xs = np.ascontiguousarray(
            np.transpose(x[:, cr * BL:(cr + 1) * BL], (0, 2, 1, 3))).astype(BF)
        h0s = np.ascontiguousarray(
            np.transpose(h0[cr * BL:(cr + 1) * BL], (1, 0, 2))).astype(BF)
        in_maps.append({
            "x": xs, "h0": h0s, "wrzh": wrzh, "wrzx": wrzx,
            "wnh": wnh, "wnx": wnx, "bneg": bneg, "ben": ben,
        })
    return in_maps


def kernel(x, h0, w_ih, w_hh, b_ih, b_hh):
    nc = _get_nc()
    in_maps = _prep_in_maps(x, h0, w_ih, w_hh, b_ih, b_hh)
    res = run_bass_kernel_spmd(nc, in_maps, list(range(NCORES)))
    ys = np.empty((T, B, COUT, L), np.float32)
    for cr in range(NCORES):
        ys[:, cr * BL:(cr + 1) * BL] = np.transpose(
            res.results[cr]["ys"].astype(np.float32), (0, 2, 1, 3))
    return ys


# revision 13
# speedup vs baseline: 1.0337x; 1.0337x over previous
"""Trainium2 Bass kernel for a convolutional GRU (nn_ConvolutionalRNN).

Reference semantics (per timestep t, torch-GRUCell-style with conv1d gates):
    gi = conv1d(x[t], w_ih) + b_ih          # [B, 3C, L]
    gh = conv1d(h,    w_hh) + b_hh          # [B, 3C, L]
    r = sigmoid(gi_r + gh_r); z = sigmoid(gi_z + gh_z)
    n = tanh(gi_n + r * gh_n)
    h = n + z * (h - n) = z*h + n*(1-z)
    ys[t] = h

Sharding: data-parallel over batch, B=16 over 8 cores -> BL=2 per core.

Kernel structure (per core):
  * The per-core batch (BL=2) is split into TWO independent recurrence
    chains (b=0, b=1) that interleave on the engines: while chain 0 runs
    its serial gate chain, chain 1's matmuls/activations fill the gaps.
  * SBUF activations are bf16 (DVE 2x perf mode); PSUM accumulates fp32.
  * One [128, nb, b, 260] "xh" buffer stack: partitions 0-63 hold h
    (written in place by the GRU update), 64-127 hold x[t].  x arrives by
    DMA into a staging ring and is copied into xh by GpSimd so the h
    update never has to synchronize against a (laggy) DMA semaphore.
    The K=3 conv over both x and h is 3 matmuls with contraction K=128
    ([whh; wih] stacked weights) per gate group:
      RZ bank [z-gates; r-gates]  <- 3 taps
      NN bank [a_n = i_n + gh_n; gh_n] <- 3 taps with block weights
  * Per chain elementwise:
      cc   = sigmoid(-RZ + bneg)            # ACT: [zc; rc], bf16
      u    = (NN[64:] + bhh_n) * cc[64:]    # DVE stt from PSUM
      npre = (NN[0:64] + b_n) - u           # DVE stt from PSUM
      n    = tanh(npre)                     # ACT
      z    = 1 - cc[0:64]                   # GpSimd
      zh   = z * h                          # GpSimd
      nzc  = n * cc[0:64]                   # DVE
      h'   = nzc + zh                       # DVE -> next xh buffer
  * Dummy matmuls into a trash PSUM bank keep the PE clock gate at 8/8.
"""

import numpy as np
import ml_dtypes
from contextlib import ExitStack

from concourse import bacc, mybir
import concourse.tile as tile
from concourse.bass_utils import run_bass_kernel_spmd

T, B, CIN, COUT, L = 128, 16, 64, 64, 256
GATES = 3 * COUT
NCORES = 8
BL = B // NCORES          # batch per core = 2 (one recurrence chain each)
NB = 4                    # xh buffer ring depth
LPAD = 260                # 2 zero cols + 256 interior + 2 zero cols
ILO, IHI = 2, 258         # interior column range
F32 = mybir.dt.float32
BF16 = mybir.dt.bfloat16
AF = mybir.ActivationFunctionType
ALU = mybir.AluOpType
BF = ml_dtypes.bfloat16


def _build_nc():
    nc = bacc.Bacc(trn_type="TRN2", target_bir_lowering=False, debug=False)

    x_d = nc.dram_tensor("x", [T, CIN, BL, L], BF16, kind="ExternalInput").ap()
    h0_d = nc.dram_tensor("h0", [COUT, BL, L], BF16, kind="ExternalInput").ap()
    wrz_d = nc.dram_tensor("wrz", [128, 3, 128], BF16, kind="ExternalInput").ap()
    wn_d = nc.dram_tensor("wn", [128, 3, 128], BF16, kind="ExternalInput").ap()
    bneg_d = nc.dram_tensor("bneg", [128, 1], F32, kind="ExternalInput").ap()
    ben_d = nc.dram_tensor("ben", [128, 1], F32, kind="ExternalInput").ap()
    ys_d = nc.dram_tensor("ys", [T, COUT, BL, L], BF16, kind="ExternalOutput").ap()

    with tile.TileContext(nc) as tc, ExitStack() as ctx:
        persist = ctx.enter_context(tc.tile_pool(name="persist", bufs=1))
        work = ctx.enter_context(tc.tile_pool(name="work", bufs=2))
        stage = ctx.enter_context(tc.tile_pool(name="stage", bufs=3))
        ps_rz = [ctx.enter_context(tc.tile_pool(name=f"ps_rz{c}", bufs=1, space="PSUM"))
                 for c in range(BL)]
        ps_nn = [ctx.enter_context(tc.tile_pool(name=f"ps_nn{c}", bufs=1, space="PSUM"))
                 for c in range(BL)]
        ps_tr = ctx.enter_context(tc.tile_pool(name="ps_tr", bufs=1, space="PSUM"))

        # --- one-time setup -------------------------------------------------
        wrz = persist.tile([128, 3, 128], BF16)
        wn = persist.tile([128, 3, 128], BF16)
        bneg = persist.tile([128, 1], F32)
        ben = persist.tile([128, 1], F32)
        nc.sync.dma_start(out=wrz[:], in_=wrz_d)
        nc.sync.dma_start(out=wn[:], in_=wn_d)
        nc.sync.dma_start(out=bneg[:], in_=bneg_d)
        nc.sync.dma_start(out=ben[:], in_=ben_d)

        xh = persist.tile([128, NB, BL, LPAD], BF16)
        nc.vector.memset(xh[:], 0.0)
        nc.sync.dma_start(out=xh[0:COUT, 0, :, ILO:IHI], in_=h0_d)
        nc.sync.dma_start(out=xh[64:128, 0, :, ILO:IHI], in_=x_d[0])
        nc.sync.dma_start(out=xh[64:128, 1, :, ILO:IHI], in_=x_d[1])

        # --- the recurrence -------------------------------------------------
        for t in range(T):
            buf, nbuf = t % NB, (t + 1) % NB
            if t + 2 < T:
                # x prefetch: DMA into a staging slot, then a GpSimd copy
                # into xh.  The copy (an engine write) is what the step
                # t+2 matmuls/h-update synchronize against -- a cheap
                # engine semaphore instead of a DMA-queue semaphore.
                xs = stage.tile([64, BL, L], BF16, tag="xs", name=f"xs_{t}")
                nc.sync.dma_start(out=xs[:], in_=x_d[t + 2])
                nc.gpsimd.tensor_copy(xh[64:128, (t + 2) % NB, :, ILO:IHI],
                                      xs[:])

            # [128, 512] fp32 = exactly one PSUM bank per tile (first 256
            # cols used) so no two chains ever share a bank.
            rz = [ps_rz[c].tile([128, 512], F32, tag=f"rz{c}", name=f"rz{c}_{t}")[:, 0:L]
                  for c in range(BL)]
            nn = [ps_nn[c].tile([128, 512], F32, tag=f"nn{c}", name=f"nn{c}_{t}")[:, 0:L]
                  for c in range(BL)]

            for c in range(BL):
                rhs = lambda k: xh[:, buf, c, 1 + k:1 + k + L]
                for k in range(3):
                    nc.tensor.matmul(rz[c][:], wrz[:, k, :], rhs(k),
                                     start=(k == 0), stop=(k == 2))
                for k in range(3):
                    nc.tensor.matmul(nn[c][:], wn[:, k, :], rhs(k),
                                     start=(k == 0), stop=(k == 2))

            cc, u, npre, n, z, zh, nzc = ({} for _ in range(7))
            for c in range(BL):
                # ACT: sigma- per chain
                cc[c] = work.tile([128, L], BF16, tag=f"cc{c}", name=f"cc{c}_{t}")
                nc.scalar.activation(cc[c][:], rz[c][:], AF.Sigmoid,
                                     bias=bneg[:], scale=-1.0)

                # DVE; u and npre both read straight from PSUM so only the
                # sigma- is upstream of them on the critical chain
                u[c] = work.tile([64, L], BF16, tag=f"u{c}", name=f"u{c}_{t}")
                nc.vector.scalar_tensor_tensor(u[c][:], nn[c][64:128, :],
                                               ben[64:128], cc[c][64:128, :],
                                               op0=ALU.add, op1=ALU.mult)
                npre[c] = work.tile([64, L], BF16, tag=f"npre{c}", name=f"npre{c}_{t}")
                nc.vector.scalar_tensor_tensor(npre[c][:], nn[c][0:64, :],
                                               ben[0:64], u[c][:],
                                               op0=ALU.add, op1=ALU.subtract)

                n[c] = work.tile([64, L], BF16, tag=f"n{c}", name=f"n{c}_{t}")
                nc.scalar.activation(n[c][:], npre[c][:], AF.Tanh)

                # GpSimd (off the critical chain): z = 1-zc, zh = z*h
                z[c] = work.tile([64, L], BF16, tag=f"z{c}", name=f"z{c}_{t}")
                nc.gpsimd.tensor_scalar(z[c][:], cc[c][0:64, :], -1.0, 1.0,
                                        op0=ALU.mult, op1=ALU.add)
                zh[c] = work.tile([64, L], BF16, tag=f"zh{c}", name=f"zh{c}_{t}")
                nc.gpsimd.tensor_tensor(zh[c][:], z[c][:],
                                        xh[0:64, buf, c, ILO:IHI], op=ALU.mult)
                nzc[c] = work.tile([64, L], BF16, tag=f"nzc{c}", name=f"nzc{c}_{t}")
                nc.vector.tensor_mul(nzc[c][:], n[c][:], cc[c][0:64, :])
                nc.vector.tensor_add(xh[0:64, nbuf, c, ILO:IHI], nzc[c][:],
                                     zh[c][:])

            nc.sync.dma_start(out=ys_d[t], in_=xh[0:64, nbuf, :, ILO:IHI])

            # HAM-warming dummies: keep the PE busy through the elementwise
            # tail so the clock gate stays at 8/8 (matmuls run 2x faster).
            tr = ps_tr.tile([128, 512], F32, tag="tr", name=f"tr_{t}")
            for dk in range(4):
                nc.tensor.matmul(tr[:, 0:384], wrz[:, dk % 3, :],
                                 wn[:, :, :], start=True, stop=True)

    nc.compile()
    return nc


_NC = None


def _get_nc():
    global _NC
    if _NC is None:
        _NC = _build_nc()
    return _NC


def _prep_in_maps(x, h0, w_ih, w_hh, b_ih, b_hh):
    w_ih = np.asarray(w_ih, np.float32)   # [192, 64, 3] (r, z, n gates)
    w_hh = np.asarray(w_hh, np.float32)
    b = np.asarray(b_ih, np.float32) + np.asarray(b_hh, np.float32)
    bhh = np.asarray(b_hh, np.float32)

    # RZ weights: rows 0:64 = h-side (whh), 64:128 = x-side (wih);
    # cols 0:64 = z gates, 64:128 = r gates
    wrz = np.zeros((128, 3, 128), np.float32)
    for k in range(3):
        wrz[0:64, k, 0:64] = w_hh[64:128, :, k].T
        wrz[0:64, k, 64:128] = w_hh[0:64, :, k].T
        wrz[64:128, k, 0:64] = w_ih[64:128, :, k].T
        wrz[64:128, k, 64:128] = w_ih[0:64, :, k].T
    # NN weights: cols 0:64 = a_n (h+x), cols 64:128 = gh_n (h only)
    wn = np.zeros((128, 3, 128), np.float32)
    for k in range(3):
        wn[0:64, k, 0:64] = w_hh[128:192, :, k].T
        wn[64:128, k, 0:64] = w_ih[128:192, :, k].T
        wn[0:64, k, 64:128] = w_hh[128:192, :, k].T

    bneg = np.zeros((128, 1), np.float32)
    bneg[0:64, 0] = -b[64:128]   # -b_z
    bneg[64:128, 0] = -b[0:64]   # -b_r
    ben = np.zeros((128, 1), np.float32)
    ben[0:64, 0] = b[128:192]            # b_ih_n + b_hh_n
    ben[64:128, 0] = bhh[128:192]        # b_hh_n

    wrz = wrz.astype(BF)
    wn = wn.astype(BF)
    x = np.asarray(x, np.float32)
    h0 = np.asarray(h0, np.float32)
    in_maps = []
    for cr in range(NCORES):
        xs = np.ascontiguousarray(
            np.transpose(x[:, cr * BL:(cr + 1) * BL], (0, 2, 1, 3))).astype(BF)
        h0s = np.ascontiguousarray(
            np.transpose(h0[cr * BL:(cr + 1) * BL], (1, 0, 2))).astype(BF)
        in_maps.append({
            "x": xs, "h0": h0s, "wrz": wrz, "wn": wn,
            "bneg": bneg, "ben": ben,
        })
    return in_maps


def kernel(x, h0, w_ih, w_hh, b_ih, b_hh):
    nc = _get_nc()
    in_maps = _prep_in_maps(x, h0, w_ih, w_hh, b_ih, b_hh)
    res = run_bass_kernel_spmd(nc, in_maps, list(range(NCORES)))
    ys = np.empty((T, B, COUT, L), np.float32)
    for cr in range(NCORES):
        ys[:, cr * BL:(cr + 1) * BL] = np.transpose(
            res.results[cr]["ys"].astype(np.float32), (0, 2, 1, 3))
    return ys


# revision 15
# speedup vs baseline: 1.2622x; 1.2210x over previous
"""Trainium2 Bass kernel for a convolutional GRU (nn_ConvolutionalRNN).

Reference semantics (per timestep t, torch-GRUCell-style with conv1d gates):
    gi = conv1d(x[t], w_ih) + b_ih          # [B, 3C, L]
    gh = conv1d(h,    w_hh) + b_hh          # [B, 3C, L]
    r = sigmoid(gi_r + gh_r); z = sigmoid(gi_z + gh_z)
    n = tanh(gi_n + r * gh_n)
    h = n + z * (h - n) = z*h + n*(1-z)
    ys[t] = h

Sharding: data-parallel over batch, B=16 over 8 cores -> BL=2 per core.

Kernel structure (per core):
  * The per-core batch (BL=2) is split into TWO independent recurrence
    chains (b=0, b=1) that interleave on the engines: while chain 0 runs
    its serial gate chain, chain 1's matmuls/activations fill the gaps.
  * SBUF activations are bf16 (DVE 2x perf mode); PSUM accumulates fp32.
  * One [128, nb, b, 260] "xh" buffer stack: partitions 0-63 hold h
    (written in place by the GRU update), 64-127 hold x[t].  x arrives by
    DMA into a staging ring and is copied into xh by GpSimd so the h
    update never has to synchronize against a (laggy) DMA semaphore.
    The K=3 conv over both x and h is 3 matmuls with contraction K=128
    ([whh; wih] stacked weights) per gate group:
      RZ bank [z-gates; r-gates]  <- 3 taps
      NN bank [a_n = i_n + gh_n; gh_n] <- 3 taps with block weights
  * Per chain elementwise:
      cc   = sigmoid(-RZ + bneg)            # ACT: [zc; rc], bf16
      u    = (NN[64:] + bhh_n) * cc[64:]    # DVE stt from PSUM
      npre = (NN[0:64] + b_n) - u           # DVE stt from PSUM
      n    = tanh(npre)                     # ACT
      z    = 1 - cc[0:64]                   # GpSimd
      zh   = z * h                          # GpSimd
      nzc  = n * cc[0:64]                   # DVE
      h'   = nzc + zh                       # DVE -> next xh buffer
  * Dummy matmuls into a trash PSUM bank keep the PE clock gate at 8/8.
"""

import numpy as np
import ml_dtypes
from contextlib import ExitStack

from concourse import bacc, mybir
import concourse.tile as tile
from concourse.bass_utils import run_bass_kernel_spmd

T, B, CIN, COUT, L = 128, 16, 64, 64, 256
GATES = 3 * COUT
NCORES = 8
BL = B // NCORES          # batch per core = 2 (one recurrence chain each)
NB = 6                    # xh buffer ring depth
LPAD = 260                # 2 zero cols + 256 interior + 2 zero cols
ILO, IHI = 2, 258         # interior column range
F32 = mybir.dt.float32
BF16 = mybir.dt.bfloat16
AF = mybir.ActivationFunctionType
ALU = mybir.AluOpType
BF = ml_dtypes.bfloat16


def _build_nc():
    nc = bacc.Bacc(trn_type="TRN2", target_bir_lowering=False, debug=False)

    x_d = nc.dram_tensor("x", [T, CIN, BL, L], BF16, kind="ExternalInput").ap()
    h0_d = nc.dram_tensor("h0", [COUT, BL, L], BF16, kind="ExternalInput").ap()
    wrz_d = nc.dram_tensor("wrz", [128, 3, 128], BF16, kind="ExternalInput").ap()
    wn_d = nc.dram_tensor("wn", [128, 3, 128], BF16, kind="ExternalInput").ap()
    bneg_d = nc.dram_tensor("bneg", [128, 1], F32, kind="ExternalInput").ap()
    ben_d = nc.dram_tensor("ben", [128, 1], F32, kind="ExternalInput").ap()
    ys_d = nc.dram_tensor("ys", [T, COUT, BL, L], BF16, kind="ExternalOutput").ap()

    with tile.TileContext(nc) as tc, ExitStack() as ctx:
        persist = ctx.enter_context(tc.tile_pool(name="persist", bufs=1))
        work = ctx.enter_context(tc.tile_pool(name="work", bufs=2))
        ps_rz = [ctx.enter_context(tc.tile_pool(name=f"ps_rz{c}", bufs=1, space="PSUM"))
                 for c in range(BL)]
        ps_nn = [ctx.enter_context(tc.tile_pool(name=f"ps_nn{c}", bufs=1, space="PSUM"))
                 for c in range(BL)]
        ps_tr = ctx.enter_context(tc.tile_pool(name="ps_tr", bufs=1, space="PSUM"))

        # --- one-time setup -------------------------------------------------
        wrz = persist.tile([128, 3, 128], BF16)
        wn = persist.tile([128, 3, 128], BF16)
        bneg = persist.tile([128, 1], F32)
        ben = persist.tile([128, 1], F32)
        nc.sync.dma_start(out=wrz[:], in_=wrz_d)
        nc.sync.dma_start(out=wn[:], in_=wn_d)
        nc.sync.dma_start(out=bneg[:], in_=bneg_d)
        nc.sync.dma_start(out=ben[:], in_=ben_d)

        xh = persist.tile([128, NB, BL, LPAD], BF16)
        nc.vector.memset(xh[:], 0.0)
        nc.sync.dma_start(out=xh[0:COUT, 0, :, ILO:IHI], in_=h0_d)
        nc.sync.dma_start(out=xh[64:128, 0, :, ILO:IHI], in_=x_d[0])
        nc.sync.dma_start(out=xh[64:128, 1, :, ILO:IHI], in_=x_d[1])

        # --- the recurrence -------------------------------------------------
        for t in range(T):
            buf, nbuf = t % NB, (t + 1) % NB
            if t + 2 < T:
                # prefetch distance 2 so the load never trails the chain
                nc.sync.dma_start(out=xh[64:128, (t + 2) % NB, :, ILO:IHI],
                                  in_=x_d[t + 2])

            # [128, 512] fp32 = exactly one PSUM bank per tile (first 256
            # cols used) so no two chains ever share a bank.
            rz = [ps_rz[c].tile([128, 512], F32, tag=f"rz{c}", name=f"rz{c}_{t}")[:, 0:L]
                  for c in range(BL)]
            nn = [ps_nn[c].tile([128, 512], F32, tag=f"nn{c}", name=f"nn{c}_{t}")[:, 0:L]
                  for c in range(BL)]

            for c in range(BL):
                rhs = lambda k: xh[:, buf, c, 1 + k:1 + k + L]
                for k in range(3):
                    nc.tensor.matmul(rz[c][:], wrz[:, k, :], rhs(k),
                                     start=(k == 0), stop=(k == 2))
                for k in range(3):
                    nc.tensor.matmul(nn[c][:], wn[:, k, :], rhs(k),
                                     start=(k == 0), stop=(k == 2))

            cc, u, npre, n, z, zh, nzc = ({} for _ in range(7))
            for c in range(BL):
                # ACT: sigma- per chain
                cc[c] = work.tile([128, L], BF16, tag=f"cc{c}", name=f"cc{c}_{t}")
                nc.scalar.activation(cc[c][:], rz[c][:], AF.Sigmoid,
                                     bias=bneg[:], scale=-1.0)

                # DVE; u and npre both read straight from PSUM so only the
                # sigma- is upstream of them on the critical chain
                u[c] = work.tile([64, L], BF16, tag=f"u{c}", name=f"u{c}_{t}")
                nc.vector.scalar_tensor_tensor(u[c][:], nn[c][64:128, :],
                                               ben[64:128], cc[c][64:128, :],
                                               op0=ALU.add, op1=ALU.mult)
                npre[c] = work.tile([64, L], BF16, tag=f"npre{c}", name=f"npre{c}_{t}")
                nc.vector.scalar_tensor_tensor(npre[c][:], nn[c][0:64, :],
                                               ben[0:64], u[c][:],
                                               op0=ALU.add, op1=ALU.subtract)

                n[c] = work.tile([64, L], BF16, tag=f"n{c}", name=f"n{c}_{t}")
                nc.scalar.activation(n[c][:], npre[c][:], AF.Tanh)

                # GpSimd (off the critical chain): z = 1-zc, zh = z*h
                z[c] = work.tile([64, L], BF16, tag=f"z{c}", name=f"z{c}_{t}")
                nc.gpsimd.tensor_scalar(z[c][:], cc[c][0:64, :], -1.0, 1.0,
                                        op0=ALU.mult, op1=ALU.add)
                zh[c] = work.tile([64, L], BF16, tag=f"zh{c}", name=f"zh{c}_{t}")
                nc.gpsimd.tensor_tensor(zh[c][:], z[c][:],
                                        xh[0:64, buf, c, ILO:IHI], op=ALU.mult)
                nzc[c] = work.tile([64, L], BF16, tag=f"nzc{c}", name=f"nzc{c}_{t}")
                nc.vector.tensor_mul(nzc[c][:], n[c][:], cc[c][0:64, :])
                nc.vector.tensor_add(xh[0:64, nbuf, c, ILO:IHI], nzc[c][:],
                                     zh[c][:])

            nc.sync.dma_start(out=ys_d[t], in_=xh[0:64, nbuf, :, ILO:IHI])

            # HAM-warming dummies: keep the PE busy through the elementwise
            # tail so the clock gate stays at 8/8 (matmuls run 2x faster).
            tr = ps_tr.tile([128, 512], F32, tag="tr", name=f"tr_{t}")
            for dk in range(8):
                nc.tensor.matmul(tr[:, 0:384], wrz[:, dk % 3, :],
                                 wn[:, :, :], start=True, stop=True)

    nc.compile()
    return nc


_NC = None


def _get_nc():
    global _NC
    if _NC is None:
        _NC = _build_nc()
    return _NC


def _prep_in_maps(x, h0, w_ih, w_hh, b_ih, b_hh):
    w_ih = np.asarray(w_ih, np.float32)   # [192, 64, 3] (r, z, n gates)
    w_hh = np.asarray(w_hh, np.float32)
    b = np.asarray(b_ih, np.float32) + np.asarray(b_hh, np.float32)
    bhh = np.asarray(b_hh, np.float32)

    # RZ weights: rows 0:64 = h-side (whh), 64:128 = x-side (wih);
    # cols 0:64 = z gates, 64:128 = r gates
    wrz = np.zeros((128, 3, 128), np.float32)
    for k in range(3):
        wrz[0:64, k, 0:64] = w_hh[64:128, :, k].T
        wrz[0:64, k, 64:128] = w_hh[0:64, :, k].T
        wrz[64:128, k, 0:64] = w_ih[64:128, :, k].T
        wrz[64:128, k, 64:128] = w_ih[0:64, :, k].T
    # NN weights: cols 0:64 = a_n (h+x), cols 64:128 = gh_n (h only)
    wn = np.zeros((128, 3, 128), np.float32)
    for k in range(3):
        wn[0:64, k, 0:64] = w_hh[128:192, :, k].T
        wn[64:128, k, 0:64] = w_ih[128:192, :, k].T
        wn[0:64, k, 64:128] = w_hh[128:192, :, k].T

    bneg = np.zeros((128, 1), np.float32)
    bneg[0:64, 0] = -b[64:128]   # -b_z
    bneg[64:128, 0] = -b[0:64]   # -b_r
    ben = np.zeros((128, 1), np.float32)
    ben[0:64, 0] = b[128:192]            # b_ih_n + b_hh_n
    ben[64:128, 0] = bhh[128:192]        # b_hh_n

    wrz = wrz.astype(BF)
    wn = wn.astype(BF)
    x = np.asarray(x, np.float32)
    h0 = np.asarray(h0, np.float32)
    in_maps = []
    for cr in range(NCORES):
        xs = np.ascontiguousarray(
            np.transpose(x[:, cr * BL:(cr + 1) * BL], (0, 2, 1, 3))).astype(BF)
        h0s = np.ascontiguousarray(
            np.transpose(h0[cr * BL:(cr + 1) * BL], (1, 0, 2))).astype(BF)
        in_maps.append({
            "x": xs, "h0": h0s, "wrz": wrz, "wn": wn,
            "bneg": bneg, "ben": ben,
        })
    return in_maps


def kernel(x, h0, w_ih, w_hh, b_ih, b_hh):
    nc = _get_nc()
    in_maps = _prep_in_maps(x, h0, w_ih, w_hh, b_ih, b_hh)
    res = run_bass_kernel_spmd(nc, in_maps, list(range(NCORES)))
    ys = np.empty((T, B, COUT, L), np.float32)
    for cr in range(NCORES):
        ys[:, cr * BL:(cr + 1) * BL] = np.transpose(
            res.results[cr]["ys"].astype(np.float32), (0, 2, 1, 3))
    return ys


# revision 17
# speedup vs baseline: 1.2646x; 1.0019x over previous
"""Trainium2 Bass kernel for a convolutional GRU (nn_ConvolutionalRNN).

Reference semantics (per timestep t, torch-GRUCell-style with conv1d gates):
    gi = conv1d(x[t], w_ih) + b_ih          # [B, 3C, L]
    gh = conv1d(h,    w_hh) + b_hh          # [B, 3C, L]
    r = sigmoid(gi_r + gh_r); z = sigmoid(gi_z + gh_z)
    n = tanh(gi_n + r * gh_n)
    h = n + z * (h - n) = z*h + n*(1-z)
    ys[t] = h

Sharding: data-parallel over batch, B=16 over 8 cores -> BL=2 per core.

Kernel structure (per core):
  * The per-core batch (BL=2) is split into TWO independent recurrence
    chains (b=0, b=1) that interleave on the engines: while chain 0 runs
    its serial gate chain, chain 1's matmuls/activations fill the gaps.
  * SBUF activations are bf16 (DVE 2x perf mode); PSUM accumulates fp32.
  * One [128, nb, b, 260] "xh" buffer stack: partitions 0-63 hold h
    (written in place by the GRU update), 64-127 hold x[t].  x arrives by
    DMA into a staging ring and is copied into xh by GpSimd so the h
    update never has to synchronize against a (laggy) DMA semaphore.
    The K=3 conv over both x and h is 3 matmuls with contraction K=128
    ([whh; wih] stacked weights) per gate group:
      RZ bank [z-gates; r-gates]  <- 3 taps
      NN bank [a_n = i_n + gh_n; gh_n] <- 3 taps with block weights
  * Per chain elementwise:
      cc   = sigmoid(-RZ + bneg)            # ACT: [zc; rc], bf16
      u    = (NN[64:] + bhh_n) * cc[64:]    # DVE stt from PSUM
      npre = (NN[0:64] + b_n) - u           # DVE stt from PSUM
      n    = tanh(npre)                     # ACT
      z    = 1 - cc[0:64]                   # GpSimd
      zh   = z * h                          # GpSimd
      nzc  = n * cc[0:64]                   # DVE
      h'   = nzc + zh                       # DVE -> next xh buffer
  * Dummy matmuls into a trash PSUM bank keep the PE clock gate at 8/8.
"""

import numpy as np
import ml_dtypes
from contextlib import ExitStack

from concourse import bacc, mybir
import concourse.tile as tile
from concourse.bass_utils import run_bass_kernel_spmd

T, B, CIN, COUT, L = 128, 16, 64, 64, 256
GATES = 3 * COUT
NCORES = 8
BL = B // NCORES          # batch per core = 2 (one recurrence chain each)
NB = 6                    # xh buffer ring depth
LPAD = 260                # 2 zero cols + 256 interior + 2 zero cols
ILO, IHI = 2, 258         # interior column range
F32 = mybir.dt.float32
BF16 = mybir.dt.bfloat16
AF = mybir.ActivationFunctionType
ALU = mybir.AluOpType
BF = ml_dtypes.bfloat16


def _build_nc():
    nc = bacc.Bacc(trn_type="TRN2", target_bir_lowering=False, debug=False)

    x_d = nc.dram_tensor("x", [T, CIN, BL, L], BF16, kind="ExternalInput").ap()
    h0_d = nc.dram_tensor("h0", [COUT, BL, L], BF16, kind="ExternalInput").ap()
    wrz_d = nc.dram_tensor("wrz", [128, 3, 128], BF16, kind="ExternalInput").ap()
    wn_d = nc.dram_tensor("wn", [128, 3, 128], BF16, kind="ExternalInput").ap()
    bneg_d = nc.dram_tensor("bneg", [128, 1], F32, kind="ExternalInput").ap()
    ben_d = nc.dram_tensor("ben", [128, 1], F32, kind="ExternalInput").ap()
    ys_d = nc.dram_tensor("ys", [T, COUT, BL, L], BF16, kind="ExternalOutput").ap()

    with tile.TileContext(nc) as tc, ExitStack() as ctx:
        persist = ctx.enter_context(tc.tile_pool(name="persist", bufs=1))
        work = ctx.enter_context(tc.tile_pool(name="work", bufs=2))
        ps_rz = [ctx.enter_context(tc.tile_pool(name=f"ps_rz{c}", bufs=1, space="PSUM"))
                 for c in range(BL)]
        ps_nn = [ctx.enter_context(tc.tile_pool(name=f"ps_nn{c}", bufs=1, space="PSUM"))
                 for c in range(BL)]
        ps_tr = ctx.enter_context(tc.tile_pool(name="ps_tr", bufs=1, space="PSUM"))

        # --- one-time setup -------------------------------------------------
        wrz = persist.tile([128, 3, 128], BF16)
        wn = persist.tile([128, 3, 128], BF16)
        bneg = persist.tile([128, 1], F32)
        ben = persist.tile([128, 1], F32)
        nc.sync.dma_start(out=wrz[:], in_=wrz_d)
        nc.sync.dma_start(out=wn[:], in_=wn_d)
        nc.sync.dma_start(out=bneg[:], in_=bneg_d)
        nc.sync.dma_start(out=ben[:], in_=ben_d)

        xh = persist.tile([128, NB, BL, LPAD], BF16)
        nc.vector.memset(xh[:], 0.0)
        nc.sync.dma_start(out=xh[0:COUT, 0, :, ILO:IHI], in_=h0_d)
        for i0 in range(4):
            nc.sync.dma_start(out=xh[64:128, i0, :, ILO:IHI], in_=x_d[i0])

        # --- the recurrence -------------------------------------------------
        for t in range(T):
            buf, nbuf = t % NB, (t + 1) % NB
            if t + 4 < T:
                # prefetch distance 4: the DMA write into slot (t+4)%NB
                # completes well before the h-update of step t+3 touches
                # that slot, so the h' add never stalls on a DMA semaphore
                nc.sync.dma_start(out=xh[64:128, (t + 4) % NB, :, ILO:IHI],
                                  in_=x_d[t + 4])

            # [128, 512] fp32 = exactly one PSUM bank per tile (first 256
            # cols used) so no two chains ever share a bank.
            rz = [ps_rz[c].tile([128, 512], F32, tag=f"rz{c}", name=f"rz{c}_{t}")[:, 0:L]
                  for c in range(BL)]
            nn = [ps_nn[c].tile([128, 512], F32, tag=f"nn{c}", name=f"nn{c}_{t}")[:, 0:L]
                  for c in range(BL)]

            for c in range(BL):
                rhs = lambda k: xh[:, buf, c, 1 + k:1 + k + L]
                for k in range(3):
                    nc.tensor.matmul(rz[c][:], wrz[:, k, :], rhs(k),
                                     start=(k == 0), stop=(k == 2))
                for k in range(3):
                    nc.tensor.matmul(nn[c][:], wn[:, k, :], rhs(k),
                                     start=(k == 0), stop=(k == 2))

            cc, u, npre, n, z, zh, nzc = ({} for _ in range(7))
            for c in range(BL):
                # ACT: sigma- per chain
                cc[c] = work.tile([128, L], BF16, tag=f"cc{c}", name=f"cc{c}_{t}")
                nc.scalar.activation(cc[c][:], rz[c][:], AF.Sigmoid,
                                     bias=bneg[:], scale=-1.0)

                # DVE; u and npre both read straight from PSUM so only the
                # sigma- is upstream of them on the critical chain
                u[c] = work.tile([64, L], BF16, tag=f"u{c}", name=f"u{c}_{t}")
                nc.vector.scalar_tensor_tensor(u[c][:], nn[c][64:128, :],
                                               ben[64:128], cc[c][64:128, :],
                                               op0=ALU.add, op1=ALU.mult)
                npre[c] = work.tile([64, L], BF16, tag=f"npre{c}", name=f"npre{c}_{t}")
                nc.vector.scalar_tensor_tensor(npre[c][:], nn[c][0:64, :],
                                               ben[0:64], u[c][:],
                                               op0=ALU.add, op1=ALU.subtract)

                n[c] = work.tile([64, L], BF16, tag=f"n{c}", name=f"n{c}_{t}")
                nc.scalar.activation(n[c][:], npre[c][:], AF.Tanh)

                # GpSimd (off the critical chain): z = 1-zc, zh = z*h
                z[c] = work.tile([64, L], BF16, tag=f"z{c}", name=f"z{c}_{t}")
                nc.gpsimd.tensor_scalar(z[c][:], cc[c][0:64, :], -1.0, 1.0,
                                        op0=ALU.mult, op1=ALU.add)
                zh[c] = work.tile([64, L], BF16, tag=f"zh{c}", name=f"zh{c}_{t}")
                nc.gpsimd.tensor_tensor(zh[c][:], z[c][:],
                                        xh[0:64, buf, c, ILO:IHI], op=ALU.mult)
                nzc[c] = work.tile([64, L], BF16, tag=f"nzc{c}", name=f"nzc{c}_{t}")
                nc.vector.tensor_mul(nzc[c][:], n[c][:], cc[c][0:64, :])
                nc.vector.tensor_add(xh[0:64, nbuf, c, ILO:IHI], nzc[c][:],
                                     zh[c][:])

            nc.sync.dma_start(out=ys_d[t], in_=xh[0:64, nbuf, :, ILO:IHI])

            # HAM-warming dummies: keep the PE busy through the elementwise
            # tail so the clock gate stays at 8/8 (matmuls run 2x faster).
            tr = ps_tr.tile([128, 512], F32, tag="tr", name=f"tr_{t}")
            for dk in range(8):
                nc.tensor.matmul(tr[:, 0:384], wrz[:, dk % 3, :],
                                 wn[:, :, :], start=True, stop=True)

    nc.compile()
    return nc


_NC = None


def _get_nc():
    global _NC
    if _NC is None:
        _NC = _build_nc()
    return _NC


def _prep_in_maps(x, h0, w_ih, w_hh, b_ih, b_hh):
    w_ih = np.asarray(w_ih, np.float32)   # [192, 64, 3] (r, z, n gates)
    w_hh = np.asarray(w_hh, np.float32)
    b = np.asarray(b_ih, np.float32) + np.asarray(b_hh, np.float32)
    bhh = np.asarray(b_hh, np.float32)

    # RZ weights: rows 0:64 = h-side (whh), 64:128 = x-side (wih);
    # cols 0:64 = z gates, 64:128 = r gates
    wrz = np.zeros((128, 3, 128), np.float32)
    for k in range(3):
        wrz[0:64, k, 0:64] = w_hh[64:128, :, k].T
        wrz[0:64, k, 64:128] = w_hh[0:64, :, k].T
        wrz[64:128, k, 0:64] = w_ih[64:128, :, k].T
        wrz[64:128, k, 64:128] = w_ih[0:64, :, k].T
    # NN weights: cols 0:64 = a_n (h+x), cols 64:128 = gh_n (h only)
    wn = np.zeros((128, 3, 128), np.float32)
    for k in range(3):
        wn[0:64, k, 0:64] = w_hh[128:192, :, k].T
        wn[64:128, k, 0:64] = w_ih[128:192, :, k].T
        wn[0:64, k, 64:128] = w_hh[128:192, :, k].T

    bneg = np.zeros((128, 1), np.float32)
    bneg[0:64, 0] = -b[64:128]   # -b_z
    bneg[64:128, 0] = -b[0:64]   # -b_r
    ben = np.zeros((128, 1), np.float32)
    ben[0:64, 0] = b[128:192]            # b_ih_n + b_hh_n
    ben[64:128, 0] = bhh[128:192]        # b_hh_n

    wrz = wrz.astype(BF)
    wn = wn.astype(BF)
    x = np.asarray(x, np.float32)
    h0 = np.asarray(h0, np.float32)
    in_maps = []
    for cr in range(NCORES):
        xs = np.ascontiguousarray(
            np.transpose(x[:, cr * BL:(cr + 1) * BL], (0, 2, 1, 3))).astype(BF)
        h0s = np.ascontiguousarray(
            np.transpose(h0[cr * BL:(cr + 1) * BL], (1, 0, 2))).astype(BF)
        in_maps.append({
            "x": xs, "h0": h0s, "wrz": wrz, "wn": wn,
            "bneg": bneg, "ben": ben,
        })
    return in_maps


def kernel(x, h0, w_ih, w_hh, b_ih, b_hh):
    nc = _get_nc()
    in_maps = _prep_in_maps(x, h0, w_ih, w_hh, b_ih, b_hh)
    res = run_bass_kernel_spmd(nc, in_maps, list(range(NCORES)))
    ys = np.empty((T, B, COUT, L), np.float32)
    for cr in range(NCORES):
        ys[:, cr * BL:(cr + 1) * BL] = np.transpose(
            res.results[cr]["ys"].astype(np.float32), (0, 2, 1, 3))
    return ys
